# revision 13
# baseline (speedup 1.0000x reference)
"""PinSageConv on 8 TRN2 NeuronCores (Bass/Tile).

Math (per node n with neighbors t=0..15):
  Y_t    = h[nb[n,t], :] @ Q_w.T (+ Q_b)
  h_agg  = sum_t (w[n,t]/sum_t w) * leaky_relu(Y_t)
  h_new  = leaky_relu([h[n] | h_agg] @ W_w.T (+ W_b))
  out    = h_new / ||h_new||_2

Sharding: data-parallel over nodes (4096 nodes/core), h table replicated,
Q/W replicated.

Per 128-node tile on device:
  - one batched indirect DMA gathers the 2048 neighbor rows (t-major in SBUF)
  - per t: PE transpose (d -> partitions), DVE copy PSUM->SBUF, 2 fp32r
    matmuls vs Q^T chunks, ACT Prelu with per-partition scale = w/sum(w)
    (exact: leaky(a*x) = a*leaky(x) for a >= 0), then PE transpose-accumulate
    into a PSUM tile, which yields h_agg^T directly (transposed operand needed
    by the W matmul).
  - W matmul from h_nodeset^T and h_agg^T chunks, Prelu, then L2 normalize
    (sum of squares on DVE, 1/sqrt via ACT Abs_reciprocal_sqrt + one Newton
    step on [128,1] scalars).
"""

import numpy as np

import concourse.bacc as bacc
import concourse.bass as bass
import concourse.mybir as mybir
from concourse.bass import IndirectOffsetOnAxis
from concourse.bass_utils import run_bass_kernel_spmd
from concourse.masks import make_identity
from concourse.tile import TileContext

F32 = mybir.dt.float32
F32R = mybir.dt.float32r
I32 = mybir.dt.int32
AX = mybir.AxisListType
OP = mybir.AluOpType
ACT = mybir.ActivationFunctionType

N_CORES = 8
P = 128
NEG_SLOPE = 0.01

# Full-problem constants (hardcoded per spec).
N_TOTAL = 500000
N_NODES = 32768
T_NB = 16
D = 256

# knobs (overridable before first kernel() call, mostly for experiments)
import os as _os

STAGE = int(_os.environ.get("KSTAGE", "9"))  # debug: cut pipeline after stage N

_PROGRAM_CACHE = {}

# set by test harness for profiling
TRACE = False
LAST_RESULT = None


def build_program(
    n_total, per_core, t_nb=T_NB, d=D, h_dim=D, o_dim=D, has_qb=False, has_wb=False
):
    """Trace + compile the per-core Bass program (identical on all cores)."""
    assert per_core % P == 0
    n_tiles = per_core // P
    kd = d // P  # k-chunks for the Q matmul (d on partitions)
    kc = (d + h_dim) // P  # k-chunks for the W matmul

    nc = bacc.Bacc("TRN2", target_bir_lowering=False, debug=False, num_devices=N_CORES)

    h = nc.dram_tensor("h", [n_total, d], F32, kind="ExternalInput")
    nb_idx = nc.dram_tensor("nb_idx", [P, n_tiles * t_nb], I32, kind="ExternalInput")
    ns_idx = nc.dram_tensor("ns_idx", [P, n_tiles], I32, kind="ExternalInput")
    nbw = nc.dram_tensor("nbw", [P, n_tiles * t_nb], F32, kind="ExternalInput")
    qt = nc.dram_tensor("qt", [P, kd, h_dim], F32R, kind="ExternalInput")
    wt = nc.dram_tensor("wt", [P, kc, o_dim], F32R, kind="ExternalInput")
    if has_qb:
        qb = nc.dram_tensor("qb", [1, h_dim], F32R, kind="ExternalInput")
    if has_wb:
        wb = nc.dram_tensor("wb", [1, o_dim], F32R, kind="ExternalInput")
    out = nc.dram_tensor("out", [per_core, o_dim], F32, kind="ExternalOutput")

    def r(ap):
        return ap.bitcast(F32R)

    with TileContext(nc) as tc:
        with (
            tc.tile_pool(name="const", bufs=1) as cpool,
            tc.tile_pool(name="gather", bufs=3) as gpool,
            tc.tile_pool(name="work", bufs=3) as wpool,
            tc.tile_pool(name="small", bufs=3) as spool,
            tc.tile_pool(name="psum", bufs=2, space="PSUM") as ppool,
            tc.tile_pool(name="psum1", bufs=1, space="PSUM") as ppool1,
        ):
            ident = cpool.tile([P, P], F32)
            make_identity(nc, ident[:])
            qt_sb = cpool.tile([P, kd * h_dim], F32R)
            nc.sync.dma_start(out=qt_sb[:], in_=qt[:])
            wt_sb = cpool.tile([P, kc * o_dim], F32R)
            nc.sync.dma_start(out=wt_sb[:], in_=wt[:])
            nb_idx_sb = cpool.tile([P, n_tiles * t_nb], I32)
            nc.sync.dma_start(out=nb_idx_sb[:], in_=nb_idx[:])
            ns_idx_sb = cpool.tile([P, n_tiles], I32)
            nc.sync.dma_start(out=ns_idx_sb[:], in_=ns_idx[:])
            nbw_sb = cpool.tile([P, n_tiles * t_nb], F32)
            nc.sync.dma_start(out=nbw_sb[:], in_=nbw[:])
            if has_qb:
                qb_sb = cpool.tile([1, h_dim], F32R)
                nc.sync.dma_start(out=qb_sb[:], in_=qb[:])
            if has_wb:
                wb_sb = cpool.tile([1, o_dim], F32R)
                nc.sync.dma_start(out=wb_sb[:], in_=wb[:])
            if has_qb or has_wb:
                ones_sb = cpool.tile([1, P], F32R)
                nc.vector.memset(ones_sb[:], 1.0)

            for i in range(n_tiles):
                # ---- gathers -------------------------------------------------
                # HW consumes ONE dynamic offset per dest partition, so each
                # indirect DMA gathers exactly 128 rows.
                g_sb = gpool.tile([P, t_nb * d], F32, tag="g")
                for t in range(t_nb):
                    nc.gpsimd.indirect_dma_start(
                        out=g_sb[:, t * d : (t + 1) * d],
                        out_offset=None,
                        in_=h[:],
                        in_offset=IndirectOffsetOnAxis(
                            ap=nb_idx_sb[:, i * t_nb + t : i * t_nb + t + 1],
                            axis=0,
                        ),
                    )
                ns_sb = gpool.tile([P, d], F32, tag="ns")
                nc.gpsimd.indirect_dma_start(
                    out=ns_sb[:],
                    out_offset=None,
                    in_=h[:],
                    in_offset=IndirectOffsetOnAxis(
                        ap=ns_idx_sb[:, i : i + 1], axis=0
                    ),
                )

                if STAGE <= 1:
                    outt1 = wpool.tile([P, o_dim], F32, tag="outt")
                    nc.vector.tensor_copy(outt1[:], g_sb[:, 0:o_dim])
                    nc.sync.dma_start(out=out[i * P : (i + 1) * P, :], in_=outt1[:])
                    continue

                # ---- normalized weights wn = w / sum_t(w) --------------------
                wsum = spool.tile([P, 1], F32, tag="wsum")
                nc.vector.tensor_reduce(
                    out=wsum[:],
                    in_=nbw_sb[:, i * t_nb : (i + 1) * t_nb],
                    axis=AX.X,
                    op=OP.add,
                )
                winv = spool.tile([P, 1], F32, tag="winv")
                nc.vector.reciprocal(winv[:], wsum[:])
                wn = spool.tile([P, t_nb], F32, tag="wn")
                nc.vector.tensor_scalar_mul(
                    wn[:], nbw_sb[:, i * t_nb : (i + 1) * t_nb], winv[:]
                )

                # ---- neighbor transform + weighted aggregation ---------------
                agg_ps = [
                    ppool1.tile([P, P], F32, tag=f"agg{c}", name=f"agg{c}")
                    for c in range(h_dim // P)
                ]
                for t in range(t_nb):
                    gt_ps = ppool.tile([P, d], F32, tag="gt")
                    for c in range(kd):
                        nc.tensor.transpose(
                            gt_ps[:, c * P : (c + 1) * P],
                            g_sb[:, t * d + c * P : t * d + (c + 1) * P],
                            ident[:],
                        )
                    gt_sb = wpool.tile([P, d], F32R, tag="gts")
                    nc.vector.tensor_copy(gt_sb[:], gt_ps[:])
                    if STAGE <= 2:
                        continue
                    y_ps = ppool.tile([P, h_dim], F32, tag="y")
                    for c in range(kd):
                        nc.tensor.matmul(
                            y_ps[:],
                            lhsT=gt_sb[:, c * P : (c + 1) * P],
                            rhs=qt_sb[:, c * h_dim : (c + 1) * h_dim],
                            start=(c == 0),
                            stop=(c == kd - 1 and not has_qb),
                        )
                    if has_qb:
                        nc.tensor.matmul(
                            y_ps[:],
                            lhsT=ones_sb[:],
                            rhs=qb_sb[:],
                            start=False,
                            stop=True,
                        )
                    if STAGE <= 3:
                        continue
                    lyw = wpool.tile([P, h_dim], F32, tag="lyw")
                    nc.scalar.activation(
                        lyw[:],
                        y_ps[:],
                        ACT.Prelu,
                        bias=0.0,
                        scale=wn[:, t : t + 1],
                        alpha=NEG_SLOPE,
                    )
                    if STAGE <= 4:
                        continue
                    for c in range(h_dim // P):
                        nc.tensor.matmul(
                            agg_ps[c][:],
                            lhsT=lyw[:, c * P : (c + 1) * P],
                            rhs=ident[:],
                            is_transpose=True,
                            start=(t == 0),
                            stop=(t == t_nb - 1),
                        )

                if STAGE == 2:
                    outt2 = wpool.tile([P, o_dim], F32, tag="outt")
                    nc.vector.tensor_copy(outt2[:], gt_sb[:].bitcast(F32))
                    nc.sync.dma_start(out=out[i * P : (i + 1) * P, :], in_=outt2[:])
                    continue
                if STAGE == 3:
                    outt3 = wpool.tile([P, o_dim], F32, tag="outt")
                    nc.vector.tensor_copy(outt3[:], y_ps[:])
                    nc.sync.dma_start(out=out[i * P : (i + 1) * P, :], in_=outt3[:])
                    continue
                if STAGE == 4:
                    outt4 = wpool.tile([P, o_dim], F32, tag="outt")
                    nc.vector.tensor_copy(outt4[:], lyw[:])
                    nc.sync.dma_start(out=out[i * P : (i + 1) * P, :], in_=outt4[:])
                    continue

                if STAGE == 5:
                    outt5 = wpool.tile([P, o_dim], F32, tag="outt")
                    nc.vector.tensor_copy(outt5[:, 0:P], agg_ps[0][:])
                    nc.vector.tensor_copy(outt5[:, P:2*P], agg_ps[1][:])
                    nc.sync.dma_start(out=out[i * P : (i + 1) * P, :], in_=outt5[:])
                    continue

                aggt_sb = wpool.tile([P, h_dim], F32R, tag="aggs")
                for c in range(h_dim // P):
                    nc.scalar.copy(
                        aggt_sb[:, c * P : (c + 1) * P], agg_ps[c][:]
                    )

                # ---- nodeset transpose ---------------------------------------
                nst_ps = ppool1.tile([P, d], F32, tag="nst")
                for c in range(kd):
                    nc.tensor.transpose(
                        nst_ps[:, c * P : (c + 1) * P],
                        ns_sb[:, c * P : (c + 1) * P],
                        ident[:],
                    )
                nst_sb = wpool.tile([P, d], F32R, tag="nsts")
                nc.vector.tensor_copy(nst_sb[:], nst_ps[:])

                if STAGE == 6:
                    outt6 = wpool.tile([P, o_dim], F32, tag="outt")
                    nc.vector.tensor_copy(outt6[:], nst_sb[:].bitcast(F32))
                    nc.sync.dma_start(out=out[i * P : (i + 1) * P, :], in_=outt6[:])
                    continue

                # ---- W matmul: [h_nodeset | h_agg] @ W^T ---------------------
                o_ps = ppool1.tile([P, o_dim], F32, tag="o")
                lhs_chunks = [
                    nst_sb[:, c * P : (c + 1) * P] for c in range(kd)
                ] + [aggt_sb[:, c * P : (c + 1) * P] for c in range(h_dim // P)]
                for c, lhs in enumerate(lhs_chunks):
                    nc.tensor.matmul(
                        o_ps[:],
                        lhsT=lhs,
                        rhs=wt_sb[:, c * o_dim : (c + 1) * o_dim],
                        start=(c == 0),
                        stop=(c == len(lhs_chunks) - 1 and not has_wb),
                    )
                if has_wb:
                    nc.tensor.matmul(
                        o_ps[:],
                        lhsT=ones_sb[:],
                        rhs=wb_sb[:],
                        start=False,
                        stop=True,
                    )

                hnew = wpool.tile([P, o_dim], F32, tag="hnew")
                nc.scalar.activation(
                    hnew[:], o_ps[:], ACT.Prelu, bias=0.0, scale=1.0, alpha=NEG_SLOPE
                )

                if STAGE == 7:
                    nc.sync.dma_start(out=out[i * P : (i + 1) * P, :], in_=hnew[:])
                    continue

                # ---- L2 normalize --------------------------------------------
                sq = wpool.tile([P, o_dim], F32, tag="sq")
                nc.scalar.square(sq[:], hnew[:])
                nrm2 = spool.tile([P, 1], F32, tag="nrm2")
                nc.vector.tensor_reduce(
                    out=nrm2[:], in_=sq[:], axis=AX.X, op=OP.add
                )
                if STAGE == 8:
                    nc.sync.dma_start(out=out[i * P : (i + 1) * P, :], in_=sq[:])
                    continue

                nrm2g = spool.tile([P, 1], F32, tag="nrm2g")
                nc.vector.tensor_scalar_max(nrm2g[:], nrm2[:], 1e-35)
                # seed y ~= 1/sqrt(s) from the ACT table, then one Newton step:
                # y' = y * (1.5 - 0.5 * s * y^2)
                y0 = spool.tile([P, 1], F32, tag="y0")
                nc.scalar.activation(y0[:], nrm2g[:], ACT.Abs_reciprocal_sqrt)
                if STAGE == 85:
                    outt85 = wpool.tile([P, o_dim], F32, tag="outt")
                    nc.scalar.mul(outt85[:], hnew[:], y0[:])
                    nc.sync.dma_start(out=out[i * P : (i + 1) * P, :], in_=outt85[:])
                    continue

                yy = spool.tile([P, 1], F32, tag="yy")
                nc.vector.tensor_tensor(
                    out=yy[:], in0=y0[:], in1=y0[:], op=OP.mult
                )
                sy = spool.tile([P, 1], F32, tag="sy")
                nc.vector.tensor_tensor(
                    out=sy[:], in0=yy[:], in1=nrm2g[:], op=OP.mult
                )
                corr = spool.tile([P, 1], F32, tag="corr")
                nc.vector.tensor_scalar(
                    out=corr[:],
                    in0=sy[:],
                    scalar1=-0.5,
                    scalar2=1.5,
                    op0=OP.mult,
                    op1=OP.add,
                )
                inv = spool.tile([P, 1], F32, tag="inv")
                nc.vector.tensor_tensor(
                    out=inv[:], in0=y0[:], in1=corr[:], op=OP.mult
                )

                outt = wpool.tile([P, o_dim], F32, tag="outt")
                nc.scalar.mul(outt[:], hnew[:], inv[:])
                nc.sync.dma_start(out=out[i * P : (i + 1) * P, :], in_=outt[:])

    nc.compile()
    return nc


def _get_program(has_qb, has_wb):
    key = (has_qb, has_wb)
    if key not in _PROGRAM_CACHE:
        _PROGRAM_CACHE[key] = build_program(
            N_TOTAL, N_NODES // N_CORES, has_qb=has_qb, has_wb=has_wb
        )
    return _PROGRAM_CACHE[key]


def _shard_host(arr_nodes, per_core, core):
    return arr_nodes[core * per_core : (core + 1) * per_core]


def _tileize(a, n_tiles, inner):
    """[per_core, inner] -> [128, n_tiles*inner] with tile i at cols i*inner."""
    return np.ascontiguousarray(
        a.reshape(n_tiles, P, inner).transpose(1, 0, 2).reshape(P, n_tiles * inner)
    )


def kernel(
    h, nodeset, nb_nodes, nb_weights, Q_w, Q_b, W_w, W_b, **_unused
) -> np.ndarray:
    global LAST_RESULT
    h = np.ascontiguousarray(np.asarray(h, dtype=np.float32))
    nodeset = np.asarray(nodeset).astype(np.int32)
    nb_nodes = np.asarray(nb_nodes).astype(np.int32)
    nb_weights = np.ascontiguousarray(np.asarray(nb_weights, dtype=np.float32))
    Q_w = np.asarray(Q_w, dtype=np.float32)
    Q_b = np.asarray(Q_b, dtype=np.float32)
    W_w = np.asarray(W_w, dtype=np.float32)
    W_b = np.asarray(W_b, dtype=np.float32)

    n_nodes = nodeset.shape[0]
    per_core = n_nodes // N_CORES
    n_tiles = per_core // P
    d = h.shape[1]
    h_dim = Q_w.shape[0]
    o_dim = W_w.shape[0]
    kd = d // P
    kc = (d + h_dim) // P

    has_qb = bool(np.any(Q_b))
    has_wb = bool(np.any(W_b))
    nc = _get_program(has_qb, has_wb)

    # Q^T / W^T pre-arranged as [128, k, out_dim] (chunk k = contraction rows
    # k*128..k*128+127).
    qt_host = np.ascontiguousarray(
        Q_w.T.reshape(kd, P, h_dim).transpose(1, 0, 2).reshape(P, kd * h_dim)
    ).reshape(P, kd, h_dim)
    wt_host = np.ascontiguousarray(
        W_w.T.reshape(kc, P, o_dim).transpose(1, 0, 2).reshape(P, kc * o_dim)
    ).reshape(P, kc, o_dim)

    in_maps = []
    for c in range(N_CORES):
        nb_c = _shard_host(nb_nodes, per_core, c)
        ns_c = _shard_host(nodeset, per_core, c)
        nw_c = _shard_host(nb_weights, per_core, c)
        im = {
            "h": h,
            "nb_idx": _tileize(nb_c, n_tiles, T_NB),
            "ns_idx": _tileize(ns_c[:, None], n_tiles, 1),
            "nbw": _tileize(nw_c, n_tiles, T_NB).astype(np.float32),
            "qt": qt_host,
            "wt": wt_host,
        }
        if has_qb:
            im["qb"] = np.ascontiguousarray(Q_b[None, :])
        if has_wb:
            im["wb"] = np.ascontiguousarray(W_b[None, :])
        in_maps.append(im)

    res = run_bass_kernel_spmd(
        nc, in_maps, list(range(N_CORES)), trace=TRACE
    )
    LAST_RESULT = res
    out = np.concatenate(
        [res.results[c]["out"] for c in range(N_CORES)], axis=0
    )
    return np.ascontiguousarray(out, dtype=np.float32)
